# revision 21
# baseline (speedup 1.0000x reference)
"""AnchorAttention distributed Bass kernel for 8 TRN2 NeuronCores.

Sharding: 2 cores per batch (core c -> batch c//2, half h = c%2). Each core
owns 2048 output rows of its batch: 128 anchor rows (h*128..h*128+128) and
1920 query rows (h*1920..), processed as 4 chunks of (32 anchors + 480
queries) = 512 rows. K/V for the batch's 256 anchors are computed ONCE per
core (not per chunk), which is 4x less projection work than the previous
per-batch-replicated scheme. No collectives: output shards are disjoint;
host reassembles.

The V matrix carries an extra ones-column so the ctx matmul also emits the
softmax denominator (psum row HD) — no separate PE sum matmuls.

All device compute uses feature-on-partition ("transposed") layouts so no
on-device transposes are needed; the host pre-transposes inputs.
"""
import sys

for _p in ("/opt/trn_rl_repo", "/root/.axon_site/_ro/trn_rl_repo"):
    if _p not in sys.path:
        sys.path.insert(0, _p)

import numpy as np
import ml_dtypes

import concourse.bass as bass
import concourse.mybir as mybir
import concourse.tile as tile
from concourse import bacc
from concourse.bass_utils import run_bass_kernel_spmd

B, N, D = 4, 4096, 1024
H, HD = 16, 64
KA = 256                   # num anchor tokens
NCORES = 8
NCH = 4                    # row chunks per core
AQ = 32                    # anchor rows per chunk
QW = 480                   # query rows per chunk
R = AQ + QW                # 512 output rows per chunk
CAQ = NCH * AQ             # 128 anchor rows per core
CQW = NCH * QW             # 1920 query rows per core
SCALE = 1.0 / float(np.sqrt(HD))

F32 = mybir.dt.float32
BF16 = mybir.dt.bfloat16
EXP = mybir.ActivationFunctionType.Exp
LN = mybir.ActivationFunctionType.Ln

BF = ml_dtypes.bfloat16


def build_graph(repeat=1, cfg=None):
    nc = bacc.Bacc("TRN2", target_bir_lowering=False, debug=False,
                   num_devices=NCORES)

    # ---- external I/O (per-core shards) ----
    xaT_e = nc.dram_tensor("xaT", [D, KA], BF16, kind="ExternalInput")
    xqT_e = nc.dram_tensor("xqT", [NCH, D, QW], BF16, kind="ExternalInput")
    xamT_e = nc.dram_tensor("xamT", [D, CAQ], BF16, kind="ExternalInput")
    wk_e = nc.dram_tensor("wk", [D, D], BF16, kind="ExternalInput")
    wv_e = nc.dram_tensor("wv", [D, D], BF16, kind="ExternalInput")
    wqt_e = nc.dram_tensor("wqt", [D, D], BF16, kind="ExternalInput")
    wq_e = nc.dram_tensor("wq", [D, D], BF16, kind="ExternalInput")
    wo_e = nc.dram_tensor("wo", [D, D], BF16, kind="ExternalInput")
    b3_e = nc.dram_tensor("b3_t", [128, 24], F32, kind="ExternalInput")
    b2_e = nc.dram_tensor("b2_r", [1, 2 * D], BF16, kind="ExternalInput")
    out_e = nc.dram_tensor("out", [NCH, R, D], BF16, kind="ExternalOutput")

    def wload(pool, ext, name):
        t = pool.tile([128, 8, D], BF16, name=name)
        nc.sync.dma_start(t[:], ext.rearrange("(o p) e -> p o e", p=128))
        return t

    with tile.TileContext(nc) as tc:
      for _rep in range(repeat):
        with tc.tile_pool(name="perm", bufs=1) as perm, \
             tc.tile_pool(name="xq_stream", bufs=8) as pxq, \
             tc.tile_pool(name="q_pool", bufs=2) as pq, \
             tc.tile_pool(name="ctx_pool", bufs=16) as pctx, \
             tc.tile_pool(name="pool_p", bufs=6) as pool_p, \
             tc.tile_pool(name="pool_rec", bufs=6) as pool_rec, \
             tc.tile_pool(name="pool_craw", bufs=6) as pool_craw, \
             tc.tile_pool(name="pool_rr", bufs=6) as pool_rr, \
             tc.tile_pool(name="pool_ot", bufs=3) as pool_ot, \
             tc.tile_pool(name="psum_proj", bufs=2, space="PSUM") as pp, \
             tc.tile_pool(name="ps_scores", bufs=2, space="PSUM") as psS, \
             tc.tile_pool(name="ps_ctx", bufs=2, space="PSUM") as psC, \
             tc.tile_pool(name="ps_out", bufs=2, space="PSUM") as psO:

            # --- DMA priority order: k-projection inputs first so the PE can
            # start on kproj while the other weights stream in ---
            wk_sb = wload(perm, wk_e, "wk_sb")
            xa_sb = perm.tile([128, 8, KA], BF16, name="xa_sb")
            nc.sync.dma_start(
                xa_sb[:], xaT_e.rearrange("(o p) f -> p o f", p=128))
            b3_sb = perm.tile([128, 24], F32)
            nc.sync.dma_start(b3_sb[:], b3_e[:])
            b2_sb = perm.tile([1, 2 * D], BF16)
            nc.sync.dma_start(b2_sb[:], b2_e[:])
            bq_sb, bk_sb, bqt_sb = b3_sb[:, 0:8], b3_sb[:, 8:16], b3_sb[:, 16:24]
            b2_bc = perm.tile([128, 2 * D], BF16)
            nc.gpsimd.partition_broadcast(b2_bc[:], b2_sb[:])
            bv_bc, bo_bc = b2_bc[:, 0:D], b2_bc[:, D:2 * D]

            wv_sb = wload(perm, wv_e, "wv_sb")
            xam_sb = perm.tile([128, 8, CAQ], BF16)
            nc.sync.dma_start(xam_sb[:], xamT_e.rearrange("(o p) f -> p o f", p=128))
            wq_sb = wload(perm, wq_e, "wq_sb")
            wqt_sb = wload(perm, wqt_e, "wqt_sb")
            xq_chunks = {}

            def load_xq(ch):
                if ch >= NCH:
                    return
                cs = []
                for dp in range(4):
                    t = pxq.tile([128, 2, QW], BF16, tag="xq", name=f"xq{ch}_{dp}")
                    nc.sync.dma_start(
                        t[:],
                        xqT_e[ch].rearrange("(o p) f -> p o f", p=128)
                        [:, dp * 2:(dp + 1) * 2, :])
                    cs.append(t)
                xq_chunks[ch] = cs

            load_xq(0)
            wo_sb = wload(perm, wo_e, "wo_sb")

            # --- once-per-core state ---
            qaT_sb = perm.tile([128, 8, CAQ], BF16)
            kT_sb = perm.tile([128, 8, KA], BF16, name="kT_sb")
            # v[a, (h, hd+1)] — last column ones => ctx matmul also produces
            # the softmax denominator in psum row HD
            v_sb = perm.tile([128, 2, H, HD + 1], BF16, name="v_sb")
            nc.vector.memset(v_sb[:, :, :, HD:HD + 1], 1.0)

            def kproj(et):
                psf = pp.tile([128, 512], F32, tag="proj", name="psk")
                ps = psf[:, :KA]
                for dt in range(8):
                    nc.tensor.matmul(
                        ps, wk_sb[:, dt, et * 128:(et + 1) * 128],
                        xa_sb[:, dt, :], start=(dt == 0), stop=(dt == 7))
                nc.scalar.add(kT_sb[:, et, :], ps, bk_sb[:, et:et + 1])

            def vproj(at, en):
                ps = pp.tile([128, 512], F32, tag="proj", name="psv")
                for dt in range(8):
                    nc.tensor.matmul(
                        ps, xa_sb[:, dt, at * 128:(at + 1) * 128],
                        wv_sb[:, dt, en * 512:(en + 1) * 512],
                        start=(dt == 0), stop=(dt == 7))
                nc.vector.tensor_add(
                    v_sb[:, at, en * 8:(en + 1) * 8, :HD],
                    ps.rearrange("p (h x) -> p h x", x=HD),
                    bv_bc[:, en * 512:(en + 1) * 512].rearrange(
                        "p (h x) -> p h x", x=HD))

            def qa_et(et):
                psf = pp.tile([128, 512], F32, tag="proj", name="psqa")
                ps = psf[:, :CAQ]
                for dt in range(8):
                    nc.tensor.matmul(
                        ps, wq_sb[:, dt, et * 128:(et + 1) * 128],
                        xam_sb[:, dt, :], start=(dt == 0), stop=(dt == 7))
                nc.vector.tensor_scalar_add(
                    qaT_sb[:, et, :], ps, bq_sb[:, et:et + 1])

            # --- per-chunk work as unit generators for software pipelining ---
            qT = {}
            ctx = {}

            def qproj_units(ch):
                """q projection for chunk ch: 9 PE unit thunks."""
                if ch >= NCH:
                    return []
                units = []

                def start(cc=ch):
                    load_xq(cc + 1)
                    qT[cc] = pq.tile([128, 8, R], BF16, tag="qT",
                                     name=f"qT{cc}")

                def qproj(et, cc=ch):
                    psf = pp.tile([128, 512], F32, tag="proj", name="psq")
                    ps = psf[:, :QW]
                    for dt in range(8):
                        nc.tensor.matmul(
                            ps, wqt_sb[:, dt, et * 128:(et + 1) * 128],
                            xq_chunks[cc][dt // 2][:, dt % 2, :],
                            start=(dt == 0), stop=(dt == 7))
                    nc.scalar.add(qT[cc][:, et, AQ:R], ps,
                                  bqt_sb[:, et:et + 1])
                    nc.vector.tensor_copy(
                        qT[cc][:, et, 0:AQ],
                        qaT_sb[:, et, cc * AQ:(cc + 1) * AQ])

                units.append(start)
                for et in range(8):
                    units.append(lambda et=et: qproj(et))
                return units

            def attn_units(ch):
                """16 head units for chunk ch."""
                ctx[ch] = [pctx.tile([128, R], BF16, tag="ctxT",
                                     name=f"ctxT{ch}_{i}") for i in range(8)]
                if cfg == "noattn":
                    def blank(et, cc=ch):
                        nc.vector.memset(ctx[cc][et][:], 0.5)
                    return [lambda et=et: blank(et) for et in range(8)]

                def head(et, par, cc=ch):
                    po = par * 64
                    h = 2 * et + par
                    kT_h = kT_sb[po:po + 64, et, :]
                    qT_h = qT[cc][po:po + 64, et, :]
                    p_t = []
                    for at in range(2):
                        ps_s = psS.tile([128, 512], F32, tag="s")
                        nc.tensor.matmul(
                            ps_s, kT_h[:, at * 128:(at + 1) * 128], qT_h,
                            start=True, stop=True, tile_position=(po, 0))
                        pt = pool_p.tile([128, 512], BF16, tag="p")
                        nc.scalar.activation(pt[:], ps_s, EXP, scale=SCALE)
                        p_t.append(pt)
                    ps_c = psC.tile([128, 512], F32, tag="c")
                    for at in range(2):
                        nc.tensor.matmul(
                            ps_c[0:HD + 1, :], v_sb[:, at, h, :],
                            p_t[at][:], start=(at == 0), stop=(at == 1))
                    if cfg == "notail":
                        nc.vector.tensor_copy(
                            ctx[cc][et][po:po + 64, :], ps_c[0:64, :])
                        return
                    if cfg == "dvetail":
                        rr = pool_rr.tile([1, 512], F32, tag="rr")
                        nc.vector.reciprocal(rr[:], ps_c[HD:HD + 1, :])
                        rec = pool_rec.tile([64, 512], F32, tag="rec")
                        nc.gpsimd.partition_broadcast(rec[:], rr[:])
                        nc.vector.tensor_mul(
                            ctx[cc][et][po:po + 64, :], ps_c[0:64, :],
                            rec[:, :])
                        return
                    # fast tail: 1/den as exp(-ln(den)) on the act engine
                    # (two ~0.33us act ops replace the 3.2us DVE reciprocal;
                    # same-engine adjacency avoids a semaphore hop)
                    lnd = pool_rr.tile([1, 512], F32, tag="lnd")
                    nc.scalar.activation(lnd[:], ps_c[HD:HD + 1, :], LN)
                    rr = pool_rr.tile([1, 512], F32, tag="rr")
                    nc.scalar.activation(rr[:], lnd[:], EXP, scale=-1.0)
                    rec = pool_rec.tile([64, 512], F32, tag="rec")
                    nc.gpsimd.partition_broadcast(rec[:], rr[:])
                    nc.vector.tensor_mul(
                        ctx[cc][et][po:po + 64, :], ps_c[0:64, :], rec[:, :])

                return [lambda et=et, par=par: head(et, par)
                        for et in range(8) for par in range(2)]

            def out_units(ch):
                """4 rt-chunk units of the output projection for chunk ch."""
                def rt_chunk(rt, cc=ch):
                    ot = pool_ot.tile([128, D], BF16, tag="ot")
                    for eo in range(2):
                        ps_o = psO.tile([128, 512], F32, tag="o")
                        for ct in range(8):
                            nc.tensor.matmul(
                                ps_o, ctx[cc][ct][:, rt * 128:(rt + 1) * 128],
                                wo_sb[:, ct, eo * 512:(eo + 1) * 512],
                                start=(ct == 0), stop=(ct == 7))
                        nc.vector.tensor_add(
                            ot[:, eo * 512:(eo + 1) * 512], ps_o,
                            bo_bc[:, eo * 512:(eo + 1) * 512])
                    nc.sync.dma_start(
                        out_e[cc, rt * 128:(rt + 1) * 128, :], ot[:])

                return [lambda rt=rt: rt_chunk(rt) for rt in range(4)]

            def weave(primary, fill):
                """Interleave fill units between primary units, evenly."""
                n, m = len(primary), len(fill)
                fi = 0
                for i, u in enumerate(primary):
                    u()
                    want = (i + 1) * m // n
                    while fi < want:
                        fill[fi]()
                        fi += 1

            # prologue: k/v projections (their DMAs land first), anchor-q
            # (needs wq), then qproj(0)
            for et in range(8):
                kproj(et)
            for at in range(2):
                for en in range(2):
                    vproj(at, en)
            for et in range(8):
                qa_et(et)
            for u in qproj_units(0):
                u()
            # pipeline: attn(ch) woven with qproj(ch+1) + out(ch-1)
            for ch in range(NCH):
                fill = qproj_units(ch + 1) + (out_units(ch - 1) if ch >= 1 else [])
                weave(attn_units(ch), fill)
            for u in out_units(NCH - 1):
                u()

    nc.compile()
    return nc


def host_prep(x, Wq, bq, Wk, bk, Wv, bv, Wqt, bqt, Wo, bo):
    """Build per-core in_maps from full inputs."""
    x = np.asarray(x, dtype=np.float32)
    bf = lambda a: np.ascontiguousarray(np.asarray(a, np.float32)).astype(BF)
    bias_t = lambda v: np.asarray(v, np.float32).reshape(8, 128).T  # [128, 8]
    b3 = np.ascontiguousarray(
        np.concatenate([bias_t(bq), bias_t(bk), bias_t(bqt)], axis=1))
    b2 = np.concatenate([np.asarray(bv, np.float32),
                         np.asarray(bo, np.float32)]).reshape(1, 2 * D)
    common = {
        "wk": bf(Wk), "wv": bf(Wv), "wqt": bf(Wqt),
        "wq": bf(Wq), "wo": bf(Wo),
        "b3_t": b3, "b2_r": b2.astype(BF),
    }
    in_maps = []
    for c in range(NCORES):
        bc, hc = c // 2, c % 2
        xa = x[bc, :KA, :]                                 # [KA, D]
        xam = x[bc, hc * CAQ:(hc + 1) * CAQ, :]            # [CAQ, D]
        xq = x[bc, KA + hc * CQW:KA + (hc + 1) * CQW, :]   # [CQW, D]
        xqT = np.ascontiguousarray(
            xq.reshape(NCH, QW, D).transpose(0, 2, 1))     # [NCH, D, QW]
        in_maps.append(dict(
            common,
            xaT=np.ascontiguousarray(xa.T).astype(BF),
            xamT=np.ascontiguousarray(xam.T).astype(BF),
            xqT=xqT.astype(BF)))
    return in_maps


def assemble(results):
    """[core][ch, r, e] shards -> full [B, N, D]."""
    out = np.empty((B, N, D), dtype=np.float32)
    for c in range(NCORES):
        bc, hc = c // 2, c % 2
        o = results[c]["out"]
        for ch in range(NCH):
            a0 = hc * CAQ + ch * AQ
            q0 = KA + hc * CQW + ch * QW
            out[bc, a0:a0 + AQ] = o[ch, :AQ]
            out[bc, q0:q0 + QW] = o[ch, AQ:]
    return out


def kernel(x, Wq, bq, Wk, bk, Wv, bv, Wqt, bqt, Wo, bo, num_anchor_tokens):
    assert int(num_anchor_tokens) == KA, f"expected {KA} anchors"
    in_maps = host_prep(x, Wq, bq, Wk, bk, Wv, bv, Wqt, bqt, Wo, bo)
    nc = build_graph()
    res = run_bass_kernel_spmd(nc, in_maps, core_ids=list(range(NCORES)))
    return assemble(res.results)


# revision 22
# speedup vs baseline: 1.1495x; 1.1495x over previous
"""AnchorAttention distributed Bass kernel for 8 TRN2 NeuronCores.

Sharding: 2 cores per batch (core c -> batch c//2, half h = c%2). Each core
owns 2048 output rows of its batch: 128 anchor rows (h*128..h*128+128) and
1920 query rows (h*1920..), processed as 4 chunks of (32 anchors + 480
queries) = 512 rows. K/V for the batch's 256 anchors are computed ONCE per
core (not per chunk), which is 4x less projection work than the previous
per-batch-replicated scheme. No collectives: output shards are disjoint;
host reassembles.

The V matrix carries an extra ones-column so the ctx matmul also emits the
softmax denominator (psum row HD) — no separate PE sum matmuls.

All device compute uses feature-on-partition ("transposed") layouts so no
on-device transposes are needed; the host pre-transposes inputs.
"""
import sys

for _p in ("/opt/trn_rl_repo", "/root/.axon_site/_ro/trn_rl_repo"):
    if _p not in sys.path:
        sys.path.insert(0, _p)

import numpy as np
import ml_dtypes

import concourse.bass as bass
import concourse.mybir as mybir
import concourse.tile as tile
from concourse import bacc
from concourse.bass_utils import run_bass_kernel_spmd

B, N, D = 4, 4096, 1024
H, HD = 16, 64
KA = 256                   # num anchor tokens
NCORES = 8
NCH = 4                    # row chunks per core
AQ = 32                    # anchor rows per chunk
QW = 480                   # query rows per chunk
R = AQ + QW                # 512 output rows per chunk
CAQ = NCH * AQ             # 128 anchor rows per core
CQW = NCH * QW             # 1920 query rows per core
SCALE = 1.0 / float(np.sqrt(HD))

F32 = mybir.dt.float32
BF16 = mybir.dt.bfloat16
EXP = mybir.ActivationFunctionType.Exp
LN = mybir.ActivationFunctionType.Ln

BF = ml_dtypes.bfloat16


def build_graph(repeat=1, cfg=None):
    nc = bacc.Bacc("TRN2", target_bir_lowering=False, debug=False,
                   num_devices=NCORES)

    # ---- external I/O (per-core shards) ----
    xaT_e = nc.dram_tensor("xaT", [D, KA], BF16, kind="ExternalInput")
    xqT_e = nc.dram_tensor("xqT", [NCH, D, QW], BF16, kind="ExternalInput")
    xamT_e = nc.dram_tensor("xamT", [D, CAQ], BF16, kind="ExternalInput")
    wk_e = nc.dram_tensor("wk", [D, D], BF16, kind="ExternalInput")
    wv_e = nc.dram_tensor("wv", [D, D], BF16, kind="ExternalInput")
    wqt_e = nc.dram_tensor("wqt", [D, D], BF16, kind="ExternalInput")
    wq_e = nc.dram_tensor("wq", [D, D], BF16, kind="ExternalInput")
    wo_e = nc.dram_tensor("wo", [D, D], BF16, kind="ExternalInput")
    b3_e = nc.dram_tensor("b3_t", [128, 24], F32, kind="ExternalInput")
    b2_e = nc.dram_tensor("b2_r", [1, 2 * D], BF16, kind="ExternalInput")
    out_e = nc.dram_tensor("out", [NCH, R, D], BF16, kind="ExternalOutput")

    def wload(pool, ext, name):
        t = pool.tile([128, 8, D], BF16, name=name)
        nc.sync.dma_start(t[:], ext.rearrange("(o p) e -> p o e", p=128))
        return t

    with tile.TileContext(nc) as tc:
      for _rep in range(repeat):
        with tc.tile_pool(name="perm", bufs=1) as perm, \
             tc.tile_pool(name="xq_stream", bufs=8) as pxq, \
             tc.tile_pool(name="q_pool", bufs=2) as pq, \
             tc.tile_pool(name="ctx_pool", bufs=16) as pctx, \
             tc.tile_pool(name="pool_p", bufs=6) as pool_p, \
             tc.tile_pool(name="pool_rec", bufs=6) as pool_rec, \
             tc.tile_pool(name="pool_craw", bufs=6) as pool_craw, \
             tc.tile_pool(name="pool_rr", bufs=6) as pool_rr, \
             tc.tile_pool(name="pool_ot", bufs=3) as pool_ot, \
             tc.tile_pool(name="psum_proj", bufs=2, space="PSUM") as pp, \
             tc.tile_pool(name="ps_scores", bufs=2, space="PSUM") as psS, \
             tc.tile_pool(name="ps_ctx", bufs=2, space="PSUM") as psC, \
             tc.tile_pool(name="ps_out", bufs=2, space="PSUM") as psO:

            # --- DMA priority order: k-projection inputs first so the PE can
            # start on kproj while the other weights stream in ---
            wk_sb = wload(perm, wk_e, "wk_sb")
            xa_sb = perm.tile([128, 8, KA], BF16, name="xa_sb")
            nc.sync.dma_start(
                xa_sb[:], xaT_e.rearrange("(o p) f -> p o f", p=128))
            b3_sb = perm.tile([128, 24], F32)
            nc.sync.dma_start(b3_sb[:], b3_e[:])
            b2_sb = perm.tile([1, 2 * D], BF16)
            nc.sync.dma_start(b2_sb[:], b2_e[:])
            bq_sb, bk_sb, bqt_sb = b3_sb[:, 0:8], b3_sb[:, 8:16], b3_sb[:, 16:24]
            b2_bc = perm.tile([128, 2 * D], BF16)
            nc.gpsimd.partition_broadcast(b2_bc[:], b2_sb[:])
            bv_bc, bo_bc = b2_bc[:, 0:D], b2_bc[:, D:2 * D]

            wv_sb = wload(perm, wv_e, "wv_sb")
            xam_sb = perm.tile([128, 8, CAQ], BF16)
            nc.sync.dma_start(xam_sb[:], xamT_e.rearrange("(o p) f -> p o f", p=128))
            wq_sb = wload(perm, wq_e, "wq_sb")
            wqt_sb = wload(perm, wqt_e, "wqt_sb")
            xq_chunks = {}

            def load_xq(ch):
                if ch >= NCH:
                    return
                cs = []
                for dp in range(4):
                    t = pxq.tile([128, 2, QW], BF16, tag="xq", name=f"xq{ch}_{dp}")
                    nc.sync.dma_start(
                        t[:],
                        xqT_e[ch].rearrange("(o p) f -> p o f", p=128)
                        [:, dp * 2:(dp + 1) * 2, :])
                    cs.append(t)
                xq_chunks[ch] = cs

            load_xq(0)
            wo_sb = wload(perm, wo_e, "wo_sb")

            # --- once-per-core state ---
            qaT_sb = perm.tile([128, 8, CAQ], BF16)
            kT_sb = perm.tile([128, 8, KA], BF16, name="kT_sb")
            # v[a, (h, hd+1)] — last column ones => ctx matmul also produces
            # the softmax denominator in psum row HD
            v_sb = perm.tile([128, 2, H, HD + 1], BF16, name="v_sb")
            nc.vector.memset(v_sb[:, :, :, HD:HD + 1], 1.0)

            def kproj(et):
                psf = pp.tile([128, 512], F32, tag="proj", name="psk")
                ps = psf[:, :KA]
                for dt in range(8):
                    nc.tensor.matmul(
                        ps, wk_sb[:, dt, et * 128:(et + 1) * 128],
                        xa_sb[:, dt, :], start=(dt == 0), stop=(dt == 7))
                nc.scalar.add(kT_sb[:, et, :], ps, bk_sb[:, et:et + 1])

            def vproj(at, en):
                ps = pp.tile([128, 512], F32, tag="proj", name="psv")
                for dt in range(8):
                    nc.tensor.matmul(
                        ps, xa_sb[:, dt, at * 128:(at + 1) * 128],
                        wv_sb[:, dt, en * 512:(en + 1) * 512],
                        start=(dt == 0), stop=(dt == 7))
                nc.vector.tensor_add(
                    v_sb[:, at, en * 8:(en + 1) * 8, :HD],
                    ps.rearrange("p (h x) -> p h x", x=HD),
                    bv_bc[:, en * 512:(en + 1) * 512].rearrange(
                        "p (h x) -> p h x", x=HD))

            def qa_et(et):
                psf = pp.tile([128, 512], F32, tag="proj", name="psqa")
                ps = psf[:, :CAQ]
                for dt in range(8):
                    nc.tensor.matmul(
                        ps, wq_sb[:, dt, et * 128:(et + 1) * 128],
                        xam_sb[:, dt, :], start=(dt == 0), stop=(dt == 7))
                nc.vector.tensor_scalar_add(
                    qaT_sb[:, et, :], ps, bq_sb[:, et:et + 1])

            # --- per-chunk work as unit generators for software pipelining ---
            qT = {}
            ctx = {}

            def qproj_units(ch):
                """q projection for chunk ch: 9 PE unit thunks."""
                if ch >= NCH:
                    return []
                units = []

                def start(cc=ch):
                    load_xq(cc + 1)
                    qT[cc] = pq.tile([128, 8, R], BF16, tag="qT",
                                     name=f"qT{cc}")

                def qproj(et, cc=ch):
                    psf = pp.tile([128, 512], F32, tag="proj", name="psq")
                    ps = psf[:, :QW]
                    for dt in range(8):
                        nc.tensor.matmul(
                            ps, wqt_sb[:, dt, et * 128:(et + 1) * 128],
                            xq_chunks[cc][dt // 2][:, dt % 2, :],
                            start=(dt == 0), stop=(dt == 7))
                    nc.scalar.add(qT[cc][:, et, AQ:R], ps,
                                  bqt_sb[:, et:et + 1])
                    nc.vector.tensor_copy(
                        qT[cc][:, et, 0:AQ],
                        qaT_sb[:, et, cc * AQ:(cc + 1) * AQ])

                units.append(start)
                for et in range(8):
                    units.append(lambda et=et: qproj(et))
                return units

            def attn_units(ch):
                """16 head units for chunk ch."""
                ctx[ch] = [pctx.tile([128, R], BF16, tag="ctxT",
                                     name=f"ctxT{ch}_{i}") for i in range(8)]
                if cfg == "noattn":
                    def blank(et, cc=ch):
                        nc.vector.memset(ctx[cc][et][:], 0.5)
                    return [lambda et=et: blank(et) for et in range(8)]

                def head(et, par, cc=ch):
                    po = par * 64
                    h = 2 * et + par
                    kT_h = kT_sb[po:po + 64, et, :]
                    qT_h = qT[cc][po:po + 64, et, :]
                    p_t = []
                    for at in range(2):
                        ps_s = psS.tile([128, 512], F32, tag="s")
                        nc.tensor.matmul(
                            ps_s, kT_h[:, at * 128:(at + 1) * 128], qT_h,
                            start=True, stop=True, tile_position=(po, 0))
                        pt = pool_p.tile([128, 512], BF16, tag="p")
                        nc.scalar.activation(pt[:], ps_s, EXP, scale=SCALE)
                        p_t.append(pt)
                    ps_c = psC.tile([128, 512], F32, tag="c")
                    for at in range(2):
                        nc.tensor.matmul(
                            ps_c[0:HD + 1, :], v_sb[:, at, h, :],
                            p_t[at][:], start=(at == 0), stop=(at == 1))
                    if cfg == "notail":
                        nc.vector.tensor_copy(
                            ctx[cc][et][po:po + 64, :], ps_c[0:64, :])
                        return
                    if cfg == "lnexp":
                        # 1/den as exp(-ln(den)) on the act engine; measured
                        # slower than the DVE reciprocal tail in-kernel
                        lnd = pool_rr.tile([1, 512], F32, tag="lnd")
                        nc.scalar.activation(lnd[:], ps_c[HD:HD + 1, :], LN)
                        rr = pool_rr.tile([1, 512], F32, tag="rr")
                        nc.scalar.activation(rr[:], lnd[:], EXP, scale=-1.0)
                    else:
                        rr = pool_rr.tile([1, 512], F32, tag="rr")
                        nc.vector.reciprocal(rr[:], ps_c[HD:HD + 1, :])
                    rec = pool_rec.tile([64, 512], F32, tag="rec")
                    nc.gpsimd.partition_broadcast(rec[:], rr[:])
                    nc.vector.tensor_mul(
                        ctx[cc][et][po:po + 64, :], ps_c[0:64, :], rec[:, :])

                return [lambda et=et, par=par: head(et, par)
                        for et in range(8) for par in range(2)]

            def out_units(ch):
                """4 rt-chunk units of the output projection for chunk ch."""
                def rt_chunk(rt, cc=ch):
                    ot = pool_ot.tile([128, D], BF16, tag="ot")
                    for eo in range(2):
                        ps_o = psO.tile([128, 512], F32, tag="o")
                        for ct in range(8):
                            nc.tensor.matmul(
                                ps_o, ctx[cc][ct][:, rt * 128:(rt + 1) * 128],
                                wo_sb[:, ct, eo * 512:(eo + 1) * 512],
                                start=(ct == 0), stop=(ct == 7))
                        nc.vector.tensor_add(
                            ot[:, eo * 512:(eo + 1) * 512], ps_o,
                            bo_bc[:, eo * 512:(eo + 1) * 512])
                    nc.sync.dma_start(
                        out_e[cc, rt * 128:(rt + 1) * 128, :], ot[:])

                return [lambda rt=rt: rt_chunk(rt) for rt in range(4)]

            def weave(primary, fill):
                """Interleave fill units between primary units, evenly."""
                n, m = len(primary), len(fill)
                fi = 0
                for i, u in enumerate(primary):
                    u()
                    want = (i + 1) * m // n
                    while fi < want:
                        fill[fi]()
                        fi += 1

            # prologue: k/v projections (their DMAs land first), anchor-q
            # (needs wq), then qproj(0)
            for et in range(8):
                kproj(et)
            for at in range(2):
                for en in range(2):
                    vproj(at, en)
            for et in range(8):
                qa_et(et)
            for u in qproj_units(0):
                u()
            # pipeline: attn(ch) woven with qproj(ch+1) + out(ch-1)
            for ch in range(NCH):
                fill = qproj_units(ch + 1) + (out_units(ch - 1) if ch >= 1 else [])
                weave(attn_units(ch), fill)
            for u in out_units(NCH - 1):
                u()

    nc.compile()
    return nc


def host_prep(x, Wq, bq, Wk, bk, Wv, bv, Wqt, bqt, Wo, bo):
    """Build per-core in_maps from full inputs."""
    x = np.asarray(x, dtype=np.float32)
    bf = lambda a: np.ascontiguousarray(np.asarray(a, np.float32)).astype(BF)
    bias_t = lambda v: np.asarray(v, np.float32).reshape(8, 128).T  # [128, 8]
    b3 = np.ascontiguousarray(
        np.concatenate([bias_t(bq), bias_t(bk), bias_t(bqt)], axis=1))
    b2 = np.concatenate([np.asarray(bv, np.float32),
                         np.asarray(bo, np.float32)]).reshape(1, 2 * D)
    common = {
        "wk": bf(Wk), "wv": bf(Wv), "wqt": bf(Wqt),
        "wq": bf(Wq), "wo": bf(Wo),
        "b3_t": b3, "b2_r": b2.astype(BF),
    }
    in_maps = []
    for c in range(NCORES):
        bc, hc = c // 2, c % 2
        xa = x[bc, :KA, :]                                 # [KA, D]
        xam = x[bc, hc * CAQ:(hc + 1) * CAQ, :]            # [CAQ, D]
        xq = x[bc, KA + hc * CQW:KA + (hc + 1) * CQW, :]   # [CQW, D]
        xqT = np.ascontiguousarray(
            xq.reshape(NCH, QW, D).transpose(0, 2, 1))     # [NCH, D, QW]
        in_maps.append(dict(
            common,
            xaT=np.ascontiguousarray(xa.T).astype(BF),
            xamT=np.ascontiguousarray(xam.T).astype(BF),
            xqT=xqT.astype(BF)))
    return in_maps


def assemble(results):
    """[core][ch, r, e] shards -> full [B, N, D]."""
    out = np.empty((B, N, D), dtype=np.float32)
    for c in range(NCORES):
        bc, hc = c // 2, c % 2
        o = results[c]["out"]
        for ch in range(NCH):
            a0 = hc * CAQ + ch * AQ
            q0 = KA + hc * CQW + ch * QW
            out[bc, a0:a0 + AQ] = o[ch, :AQ]
            out[bc, q0:q0 + QW] = o[ch, AQ:]
    return out


def kernel(x, Wq, bq, Wk, bk, Wv, bv, Wqt, bqt, Wo, bo, num_anchor_tokens):
    assert int(num_anchor_tokens) == KA, f"expected {KA} anchors"
    in_maps = host_prep(x, Wq, bq, Wk, bk, Wv, bv, Wqt, bqt, Wo, bo)
    nc = build_graph()
    res = run_bass_kernel_spmd(nc, in_maps, core_ids=list(range(NCORES)))
    return assemble(res.results)


# revision 25
# speedup vs baseline: 1.1904x; 1.0356x over previous
"""AnchorAttention distributed Bass kernel for 8 TRN2 NeuronCores.

Sharding: 2 cores per batch (core c -> batch c//2, half h = c%2). Each core
owns 2048 output rows of its batch: 128 anchor rows (h*128..h*128+128) and
1920 query rows (h*1920..), processed as 4 chunks of (32 anchors + 480
queries) = 512 rows. K/V for the batch's 256 anchors are computed ONCE per
core (not per chunk), which is 4x less projection work than the previous
per-batch-replicated scheme. No collectives: output shards are disjoint;
host reassembles.

The V matrix carries an extra ones-column so the ctx matmul also emits the
softmax denominator (psum row HD) — no separate PE sum matmuls.

All device compute uses feature-on-partition ("transposed") layouts so no
on-device transposes are needed; the host pre-transposes inputs.
"""
import sys

for _p in ("/opt/trn_rl_repo", "/root/.axon_site/_ro/trn_rl_repo"):
    if _p not in sys.path:
        sys.path.insert(0, _p)

import numpy as np
import ml_dtypes

import concourse.bass as bass
import concourse.mybir as mybir
import concourse.tile as tile
from concourse import bacc
from concourse.bass_utils import run_bass_kernel_spmd

B, N, D = 4, 4096, 1024
H, HD = 16, 64
KA = 256                   # num anchor tokens
NCORES = 8
NCH = 4                    # row chunks per core
AQ = 32                    # anchor rows per chunk
QW = 480                   # query rows per chunk
R = AQ + QW                # 512 output rows per chunk
CAQ = NCH * AQ             # 128 anchor rows per core
CQW = NCH * QW             # 1920 query rows per core
SCALE = 1.0 / float(np.sqrt(HD))

F32 = mybir.dt.float32
BF16 = mybir.dt.bfloat16
EXP = mybir.ActivationFunctionType.Exp
LN = mybir.ActivationFunctionType.Ln

BF = ml_dtypes.bfloat16


def build_graph(repeat=1, cfg=None):
    nc = bacc.Bacc("TRN2", target_bir_lowering=False, debug=False,
                   num_devices=NCORES)

    # ---- external I/O (per-core shards) ----
    xaT_e = nc.dram_tensor("xaT", [D, KA], BF16, kind="ExternalInput")
    xqT_e = nc.dram_tensor("xqT", [NCH, D, QW], BF16, kind="ExternalInput")
    xamT_e = nc.dram_tensor("xamT", [D, CAQ], BF16, kind="ExternalInput")
    wk_e = nc.dram_tensor("wk", [D, D], BF16, kind="ExternalInput")
    wv_e = nc.dram_tensor("wv", [D, D], BF16, kind="ExternalInput")
    wqt_e = nc.dram_tensor("wqt", [D, D], BF16, kind="ExternalInput")
    wq_e = nc.dram_tensor("wq", [D, D], BF16, kind="ExternalInput")
    wo_e = nc.dram_tensor("wo", [D, D], BF16, kind="ExternalInput")
    b3_e = nc.dram_tensor("b3_t", [128, 24], F32, kind="ExternalInput")
    b2_e = nc.dram_tensor("b2_r", [1, 2 * D], BF16, kind="ExternalInput")
    out_e = nc.dram_tensor("out", [NCH, R, D], BF16, kind="ExternalOutput")

    def wload(pool, ext, name):
        t = pool.tile([128, 8, D], BF16, name=name)
        nc.sync.dma_start(t[:], ext.rearrange("(o p) e -> p o e", p=128))
        return t

    with tile.TileContext(nc) as tc:
      for _rep in range(repeat):
        with tc.tile_pool(name="perm", bufs=1) as perm, \
             tc.tile_pool(name="xq_stream", bufs=8) as pxq, \
             tc.tile_pool(name="q_pool", bufs=2) as pq, \
             tc.tile_pool(name="ctx_pool", bufs=16) as pctx, \
             tc.tile_pool(name="pool_p", bufs=6) as pool_p, \
             tc.tile_pool(name="pool_rec", bufs=6) as pool_rec, \
             tc.tile_pool(name="pool_craw", bufs=6) as pool_craw, \
             tc.tile_pool(name="pool_rr", bufs=6) as pool_rr, \
             tc.tile_pool(name="pool_ot", bufs=3) as pool_ot, \
             tc.tile_pool(name="psum_proj", bufs=2, space="PSUM") as pp, \
             tc.tile_pool(name="ps_scores", bufs=2, space="PSUM") as psS, \
             tc.tile_pool(name="ps_ctx", bufs=(3 if cfg == "c3o1" else 2),
                          space="PSUM") as psC, \
             tc.tile_pool(name="ps_out", bufs=(1 if cfg == "c3o1" else 2),
                          space="PSUM") as psO:

            # --- DMA priority order: k-projection inputs first so the PE can
            # start on kproj while the other weights stream in ---
            wk_sb = wload(perm, wk_e, "wk_sb")
            xa_sb = perm.tile([128, 8, KA], BF16, name="xa_sb")
            nc.sync.dma_start(
                xa_sb[:], xaT_e.rearrange("(o p) f -> p o f", p=128))
            b3_sb = perm.tile([128, 24], F32)
            nc.sync.dma_start(b3_sb[:], b3_e[:])
            b2_sb = perm.tile([1, 2 * D], BF16)
            nc.sync.dma_start(b2_sb[:], b2_e[:])
            bq_sb, bk_sb, bqt_sb = b3_sb[:, 0:8], b3_sb[:, 8:16], b3_sb[:, 16:24]
            b2_bc = perm.tile([128, 2 * D], BF16)
            nc.gpsimd.partition_broadcast(b2_bc[:], b2_sb[:])
            bv_bc, bo_bc = b2_bc[:, 0:D], b2_bc[:, D:2 * D]

            wv_sb = wload(perm, wv_e, "wv_sb")
            xam_sb = perm.tile([128, 8, CAQ], BF16)
            nc.sync.dma_start(xam_sb[:], xamT_e.rearrange("(o p) f -> p o f", p=128))
            wq_sb = wload(perm, wq_e, "wq_sb")
            wqt_sb = wload(perm, wqt_e, "wqt_sb")
            xq_chunks = {}

            def load_xq(ch):
                if ch >= NCH:
                    return
                cs = []
                for dp in range(4):
                    t = pxq.tile([128, 2, QW], BF16, tag="xq", name=f"xq{ch}_{dp}")
                    nc.sync.dma_start(
                        t[:],
                        xqT_e[ch].rearrange("(o p) f -> p o f", p=128)
                        [:, dp * 2:(dp + 1) * 2, :])
                    cs.append(t)
                xq_chunks[ch] = cs

            load_xq(0)
            wo_sb = wload(perm, wo_e, "wo_sb")

            # --- once-per-core state ---
            qaT_sb = perm.tile([128, 8, CAQ], BF16)
            kT_sb = perm.tile([128, 8, KA], BF16, name="kT_sb")
            # v[a, (h, hd+1)] — last column ones => ctx matmul also produces
            # the softmax denominator in psum row HD
            v_sb = perm.tile([128, 2, H, HD + 1], BF16, name="v_sb")
            nc.vector.memset(v_sb[:, :, :, HD:HD + 1], 1.0)

            def kproj(et, pool=None):
                psf = (pool or pp).tile([128, 512], F32,
                                        tag=("o" if pool is not None else "proj"),
                                        name="psk")
                ps = psf[:, :KA]
                for dt in range(8):
                    nc.tensor.matmul(
                        ps, wk_sb[:, dt, et * 128:(et + 1) * 128],
                        xa_sb[:, dt, :], start=(dt == 0), stop=(dt == 7))
                nc.scalar.add(kT_sb[:, et, :], ps, bk_sb[:, et:et + 1])

            def vproj(at, en, pool=None):
                ps = (pool or pp).tile([128, 512], F32,
                                       tag=("o" if pool is not None else "proj"),
                                       name="psv")
                for dt in range(8):
                    nc.tensor.matmul(
                        ps, xa_sb[:, dt, at * 128:(at + 1) * 128],
                        wv_sb[:, dt, en * 512:(en + 1) * 512],
                        start=(dt == 0), stop=(dt == 7))
                nc.vector.tensor_add(
                    v_sb[:, at, en * 8:(en + 1) * 8, :HD],
                    ps.rearrange("p (h x) -> p h x", x=HD),
                    bv_bc[:, en * 512:(en + 1) * 512].rearrange(
                        "p (h x) -> p h x", x=HD))

            def qa_et(et, pool=None):
                psf = (pool or pp).tile([128, 512], F32,
                                        tag=("o" if pool is not None else "proj"),
                                        name="psqa")
                ps = psf[:, :CAQ]
                for dt in range(8):
                    nc.tensor.matmul(
                        ps, wq_sb[:, dt, et * 128:(et + 1) * 128],
                        xam_sb[:, dt, :], start=(dt == 0), stop=(dt == 7))
                nc.vector.tensor_scalar_add(
                    qaT_sb[:, et, :], ps, bq_sb[:, et:et + 1])

            # --- per-chunk work as unit generators for software pipelining ---
            qT = {}
            ctx = {}

            def qproj_units(ch):
                """q projection for chunk ch: 9 PE unit thunks."""
                if ch >= NCH:
                    return []
                units = []

                def start(cc=ch):
                    load_xq(cc + 1)
                    qT[cc] = pq.tile([128, 8, R], BF16, tag="qT",
                                     name=f"qT{cc}")

                def qproj(et, cc=ch):
                    psf = pp.tile([128, 512], F32, tag="proj", name="psq")
                    ps = psf[:, :QW]
                    for dt in range(8):
                        nc.tensor.matmul(
                            ps, wqt_sb[:, dt, et * 128:(et + 1) * 128],
                            xq_chunks[cc][dt // 2][:, dt % 2, :],
                            start=(dt == 0), stop=(dt == 7))
                    nc.scalar.add(qT[cc][:, et, AQ:R], ps,
                                  bqt_sb[:, et:et + 1])
                    nc.vector.tensor_copy(
                        qT[cc][:, et, 0:AQ],
                        qaT_sb[:, et, cc * AQ:(cc + 1) * AQ])

                units.append(start)
                for et in range(8):
                    units.append(lambda et=et: qproj(et))
                return units

            def attn_units(ch):
                """16 head units for chunk ch."""
                ctx[ch] = [pctx.tile([128, R], BF16, tag="ctxT",
                                     name=f"ctxT{ch}_{i}") for i in range(8)]
                if cfg == "noattn":
                    def blank(et, cc=ch):
                        nc.vector.memset(ctx[cc][et][:], 0.5)
                    return [lambda et=et: blank(et) for et in range(8)]

                def head(et, par, cc=ch):
                    po = par * 64
                    h = 2 * et + par
                    kT_h = kT_sb[po:po + 64, et, :]
                    qT_h = qT[cc][po:po + 64, et, :]
                    p_t = []
                    for at in range(2):
                        ps_s = psS.tile([128, 512], F32, tag="s")
                        nc.tensor.matmul(
                            ps_s, kT_h[:, at * 128:(at + 1) * 128], qT_h,
                            start=True, stop=True, tile_position=(po, 0))
                        pt = pool_p.tile([128, 512], BF16, tag="p")
                        nc.scalar.activation(pt[:], ps_s, EXP, scale=SCALE)
                        p_t.append(pt)
                    ps_c = psC.tile([128, 512], F32, tag="c")
                    for at in range(2):
                        nc.tensor.matmul(
                            ps_c[0:HD + 1, :], v_sb[:, at, h, :],
                            p_t[at][:], start=(at == 0), stop=(at == 1))
                    if cfg == "notail":
                        nc.vector.tensor_copy(
                            ctx[cc][et][po:po + 64, :], ps_c[0:64, :])
                        return
                    if cfg == "lnexp":
                        # 1/den as exp(-ln(den)) on the act engine; measured
                        # slower than the DVE reciprocal tail in-kernel
                        lnd = pool_rr.tile([1, 512], F32, tag="lnd")
                        nc.scalar.activation(lnd[:], ps_c[HD:HD + 1, :], LN)
                        rr = pool_rr.tile([1, 512], F32, tag="rr")
                        nc.scalar.activation(rr[:], lnd[:], EXP, scale=-1.0)
                    else:
                        rr = pool_rr.tile([1, 512], F32, tag="rr")
                        nc.vector.reciprocal(rr[:], ps_c[HD:HD + 1, :])
                    rec = pool_rec.tile([64, 512], F32, tag="rec")
                    nc.gpsimd.partition_broadcast(rec[:], rr[:])
                    eng = (nc.gpsimd if (cfg == "gpmul" and par == 1)
                           else nc.vector)
                    eng.tensor_mul(
                        ctx[cc][et][po:po + 64, :], ps_c[0:64, :], rec[:, :])

                return [lambda et=et, par=par: head(et, par)
                        for et in range(8) for par in range(2)]

            def out_units(ch):
                """4 rt-chunk units of the output projection for chunk ch."""
                def rt_chunk(rt, cc=ch):
                    ot = pool_ot.tile([128, D], BF16, tag="ot")
                    for eo in range(2):
                        ps_o = psO.tile([128, 512], F32, tag="o")
                        for ct in range(8):
                            nc.tensor.matmul(
                                ps_o, ctx[cc][ct][:, rt * 128:(rt + 1) * 128],
                                wo_sb[:, ct, eo * 512:(eo + 1) * 512],
                                start=(ct == 0), stop=(ct == 7))
                        nc.vector.tensor_add(
                            ot[:, eo * 512:(eo + 1) * 512], ps_o,
                            bo_bc[:, eo * 512:(eo + 1) * 512])
                    nc.sync.dma_start(
                        out_e[cc, rt * 128:(rt + 1) * 128, :], ot[:])

                return [lambda rt=rt: rt_chunk(rt) for rt in range(4)]

            def weave(primary, fill):
                """Interleave fill units between primary units, evenly."""
                n, m = len(primary), len(fill)
                fi = 0
                for i, u in enumerate(primary):
                    u()
                    want = (i + 1) * m // n
                    while fi < want:
                        fill[fi]()
                        fi += 1

            # prologue: k/v projections (their DMAs land first), anchor-q
            # (needs wq), then qproj(0)
            gi = 0
            for et in range(8):
                kproj(et, pool=(None if gi % 2 == 0 else psO)); gi += 1
            for at in range(2):
                for en in range(2):
                    vproj(at, en, pool=(None if gi % 2 == 0 else psO)); gi += 1
            for et in range(8):
                qa_et(et, pool=(None if gi % 2 == 0 else psO)); gi += 1
            for u in qproj_units(0):
                u()
            # pipeline: attn(ch) woven with qproj(ch+1) + out(ch-1)
            for ch in range(NCH):
                fill = qproj_units(ch + 1) + (out_units(ch - 1) if ch >= 1 else [])
                weave(attn_units(ch), fill)
            for u in out_units(NCH - 1):
                u()

    nc.compile()
    return nc


def host_prep(x, Wq, bq, Wk, bk, Wv, bv, Wqt, bqt, Wo, bo):
    """Build per-core in_maps from full inputs."""
    x = np.asarray(x, dtype=np.float32)
    bf = lambda a: np.ascontiguousarray(np.asarray(a, np.float32)).astype(BF)
    bias_t = lambda v: np.asarray(v, np.float32).reshape(8, 128).T  # [128, 8]
    b3 = np.ascontiguousarray(
        np.concatenate([bias_t(bq), bias_t(bk), bias_t(bqt)], axis=1))
    b2 = np.concatenate([np.asarray(bv, np.float32),
                         np.asarray(bo, np.float32)]).reshape(1, 2 * D)
    common = {
        "wk": bf(Wk), "wv": bf(Wv), "wqt": bf(Wqt),
        "wq": bf(Wq), "wo": bf(Wo),
        "b3_t": b3, "b2_r": b2.astype(BF),
    }
    in_maps = []
    for c in range(NCORES):
        bc, hc = c // 2, c % 2
        xa = x[bc, :KA, :]                                 # [KA, D]
        xam = x[bc, hc * CAQ:(hc + 1) * CAQ, :]            # [CAQ, D]
        xq = x[bc, KA + hc * CQW:KA + (hc + 1) * CQW, :]   # [CQW, D]
        xqT = np.ascontiguousarray(
            xq.reshape(NCH, QW, D).transpose(0, 2, 1))     # [NCH, D, QW]
        in_maps.append(dict(
            common,
            xaT=np.ascontiguousarray(xa.T).astype(BF),
            xamT=np.ascontiguousarray(xam.T).astype(BF),
            xqT=xqT.astype(BF)))
    return in_maps


def assemble(results):
    """[core][ch, r, e] shards -> full [B, N, D]."""
    out = np.empty((B, N, D), dtype=np.float32)
    for c in range(NCORES):
        bc, hc = c // 2, c % 2
        o = results[c]["out"]
        for ch in range(NCH):
            a0 = hc * CAQ + ch * AQ
            q0 = KA + hc * CQW + ch * QW
            out[bc, a0:a0 + AQ] = o[ch, :AQ]
            out[bc, q0:q0 + QW] = o[ch, AQ:]
    return out


def kernel(x, Wq, bq, Wk, bk, Wv, bv, Wqt, bqt, Wo, bo, num_anchor_tokens):
    assert int(num_anchor_tokens) == KA, f"expected {KA} anchors"
    in_maps = host_prep(x, Wq, bq, Wk, bk, Wv, bv, Wqt, bqt, Wo, bo)
    nc = build_graph()
    res = run_bass_kernel_spmd(nc, in_maps, core_ids=list(range(NCORES)))
    return assemble(res.results)
